# revision 23
# baseline (speedup 1.0000x reference)
"""Trainium2 Bass kernel for nn_Mixer2dTriU (B=1024, T=128, C=128, fp32).

Data-parallel over 8 NeuronCores: 128 batches/core, groups of 4 batches
stacked along the free dim ([128, 512] tiles).

Per-batch math (reference):
    h  = LN_{T,C}(x) * g1 + b1            (g1=ones, b1=zeros per spec)
    tm = tril(Wt) @ h + bt[:, None]
    x2 = LN_{T,C}(tm + x) * g2 + b2
    y  = gelu(x2 @ W1.T + b1v) @ W2.T + b2v
    out = x2 + y

Device design:
  - LN stats via DVE bn_stats + gpsimd partition_all_reduce (cross-partition)
    + Newton rsqrt on DVE (ACT Sqrt table is low-precision and would thrash
    the Gelu LUT table set).
  - time-mix = one batched matmul lhsT=tril(Wt).T (fp32r) + exact I@x (fp32)
    residual accumulated in PSUM; bt folded into LN2 stats + evict bias.
  - LN2 normalize fused into the PSUM->SBUF evict on ScalarE:
    x2 = tm_psum * inv2 + (bt - mu2) * inv2   (per-batch scalars).
  - PE transposes x2 -> channel-major, MLP via static-weight matmuls (fp32r),
    gelu+b1 fused on ScalarE, final residual via one scalar_tensor_tensor,
    PE transposes back, ScalarE evict, contiguous DMA store.
"""

import os
import sys

for _p in ("/opt/trn_rl_repo",):
    if _p not in sys.path and os.path.isdir(_p):
        sys.path.insert(0, _p)

import numpy as np

import concourse.bacc as bacc
import concourse.bass as bass
import concourse.mybir as mybir
from concourse import bass_isa
from concourse.bass_utils import run_bass_kernel_spmd
from concourse.tile import TileContext

B, T, C = 1024, 128, 128
NCORES = 8
BL = B // NCORES          # 128 batches per core
G = 4                     # batches per group -> free dim 512
NG = BL // G              # 32 groups
SG2 = 4                   # groups per LN2 stats batch (PSUM-limited)
EPS = 1e-5
NTC = float(T * C)        # elements per LN block
FD = G * C                # 512

F32 = mybir.dt.float32
F32R = mybir.dt.float32r
AX = mybir.AxisListType
OP = mybir.AluOpType
AF = mybir.ActivationFunctionType

# Engine for the LN1 normalize (h = (x-mu)*inv): "gpsimd" or "vector"
H_NORM_ENGINE = os.environ.get("MIXER_HNORM_ENGINE", "vector")
# Matmul dtype for the W-matmuls: "f32r" or "f32"
MM_DTYPE = os.environ.get("MIXER_MM_DTYPE", "f32r")
# How many of the 4 x2-evict blocks go to ScalarE (rest on VectorE)
X2_EVICT_ACT_BLOCKS = int(os.environ.get("MIXER_X2_ACT_BLOCKS", "3"))
# CoreSim has no Gelu LUT; this swaps in Identity for sim-only validation.
SIM_NOGELU = bool(os.environ.get("MIXER_SIM_NOGELU"))


def _bn_stats_blocks(nc, parts_ap, col0, in_flat_ap, nblk):
    """HW BNStats wants out == [128, 6] per instruction: one call per
    C-column block. parts_ap: [128, S, 6] tile AP; in_flat_ap: [128, nblk*C].
    """
    pf = parts_ap.rearrange("p s k -> p (s k)")
    for b in range(nblk):
        col = col0 + b
        nc.vector.bn_stats(
            pf[:, col * 6 : (col + 1) * 6],
            in_flat_ap[:, b * C : (b + 1) * C],
        )


def _newton_rsqrt(nc, pool, varr, n, y0, iters, tag):
    """inv = rsqrt(varr + EPS) on DVE, [128, n] tiles. varr is an SBUF AP.

    Seed y0 (python float) must satisfy |y0*sqrt(v+eps) - 1| < ~0.5 for all
    expected v; each Newton step squares the error.
    """
    weps = pool.tile([128, n], F32, tag=f"{tag}_weps")
    nc.vector.tensor_scalar(weps[:], varr, 1.0, EPS, OP.mult, OP.add)
    y = pool.tile([128, n], F32, tag=f"{tag}_y")
    # first iteration with constant seed folded in:
    # y1 = 1.5*y0 - 0.5*y0^3 * w
    nc.vector.tensor_scalar(
        y[:], weps[:], -0.5 * y0 ** 3, 1.5 * y0, OP.mult, OP.add
    )
    t = pool.tile([128, n], F32, tag=f"{tag}_t")
    for _ in range(iters - 1):
        nc.vector.tensor_tensor(t[:], y[:], y[:], OP.mult)
        nc.vector.tensor_tensor(t[:], t[:], weps[:], OP.mult)
        nc.vector.tensor_scalar(t[:], t[:], -0.5, 1.5, OP.mult, OP.add)
        nc.vector.tensor_tensor(y[:], y[:], t[:], OP.mult)
    return y


def _ln_stats_from_parts(nc, pool, parts_ap, nb, tag):
    """parts_ap: [128, nb, 6] bn_stats output (count,mean,ct*var) x (even,odd)
    with 64-element halves. Returns sums tile [128, 2*nb]: cols 0:nb = per-
    partition block sums, nb:2nb = per-partition block sum-of-squares."""
    means = parts_ap.rearrange("p s (a b) -> p s a b", a=2, b=3)[:, :, :, 1:2]
    means = means.squeeze(3)          # [128, nb, 2]
    ctvs = parts_ap.rearrange("p s (a b) -> p s a b", a=2, b=3)[:, :, :, 2:3]
    ctvs = ctvs.squeeze(3)            # [128, nb, 2]

    msum = pool.tile([128, nb], F32, tag=f"{tag}_msum")
    nc.vector.tensor_reduce(msum[:], means, axis=AX.X, op=OP.add)
    msq = pool.tile([128, nb, 2], F32, tag=f"{tag}_msq")
    nc.vector.tensor_tensor(msq[:], means, means, OP.mult)
    msqs = pool.tile([128, nb], F32, tag=f"{tag}_msqs")
    nc.vector.tensor_reduce(msqs[:], msq[:], axis=AX.X, op=OP.add)
    ctvsum = pool.tile([128, nb], F32, tag=f"{tag}_ctvs")
    nc.vector.tensor_reduce(ctvsum[:], ctvs, axis=AX.X, op=OP.add)

    sums = pool.tile([128, 2 * nb], F32, tag=f"{tag}_sums")
    nc.vector.tensor_scalar(sums[:, 0:nb], msum[:], 64.0, None, OP.mult)
    nc.vector.scalar_tensor_tensor(
        sums[:, nb : 2 * nb], msqs[:], 64.0, ctvsum[:], OP.mult, OP.add
    )
    return sums


def build_nc(apply_gb: bool) -> bass.Bass:
    nc = bacc.Bacc()

    x_in = nc.declare_dram_parameter("x_in", [BL, T, C], F32, isOutput=False)
    mmdt = F32R if MM_DTYPE == "f32r" else F32
    wmT = nc.declare_dram_parameter("wmT", [T, T], mmdt, isOutput=False)
    w1T = nc.declare_dram_parameter("w1T", [C, C], mmdt, isOutput=False)
    w2T = nc.declare_dram_parameter("w2T", [C, C], mmdt, isOutput=False)
    ident = nc.declare_dram_parameter("ident", [128, 128], F32, isOutput=False)
    btv = nc.declare_dram_parameter("btv", [T, 1], F32, isOutput=False)
    b1v = nc.declare_dram_parameter("b1v", [C, 1], F32, isOutput=False)
    b2v = nc.declare_dram_parameter("b2v", [C, 1], F32, isOutput=False)
    if apply_gb:
        g1m = nc.declare_dram_parameter("g1m", [T, C], F32, isOutput=False)
        b1m = nc.declare_dram_parameter("b1m", [T, C], F32, isOutput=False)
        g2m = nc.declare_dram_parameter("g2m", [T, C], F32, isOutput=False)
        b2m = nc.declare_dram_parameter("b2m", [T, C], F32, isOutput=False)
    y_out = nc.declare_dram_parameter("y_out", [BL, T, C], F32, isOutput=True)

    with TileContext(nc) as tc:
        with (
            tc.tile_pool(name="const", bufs=1) as cpool,
            tc.tile_pool(name="xg", bufs=NG) as p_xg,
            tc.tile_pool(name="h", bufs=3) as p_h,
            tc.tile_pool(name="x2", bufs=3) as p_x2,
            tc.tile_pool(name="x2ct", bufs=3) as p_x2ct,
            tc.tile_pool(name="gct", bufs=2) as p_gct,
            tc.tile_pool(name="outct", bufs=2) as p_outct,
            tc.tile_pool(name="outsb", bufs=3) as p_outsb,
            tc.tile_pool(name="stats", bufs=2) as p_st,
            tc.tile_pool(name="parts1", bufs=1) as p_parts1,
            tc.tile_pool(name="parts2", bufs=2) as p_parts2,
            tc.tile_pool(name="tmps", bufs=4, space="PSUM") as p_tm,
            tc.tile_pool(name="ctps", bufs=1, space="PSUM") as p_ctps,
            tc.tile_pool(name="m1ps", bufs=1, space="PSUM") as p_m1,
            tc.tile_pool(name="h2ps", bufs=1, space="PSUM") as p_h2,
            tc.tile_pool(name="otps", bufs=1, space="PSUM") as p_ot,
        ):
            # ---- constants ----
            wmT_sb = cpool.tile([T, T], mmdt)
            nc.sync.dma_start(wmT_sb[:], wmT[:])
            w1T_sb = cpool.tile([C, C], mmdt)
            nc.sync.dma_start(w1T_sb[:], w1T[:])
            w2T_sb = cpool.tile([C, C], mmdt)
            nc.sync.dma_start(w2T_sb[:], w2T[:])
            ident_sb = cpool.tile([128, 128], F32)
            nc.sync.dma_start(ident_sb[:], ident[:])
            btv_sb = cpool.tile([T, 1], F32)
            nc.sync.dma_start(btv_sb[:], btv[:])
            b1v_sb = cpool.tile([C, 1], F32)
            nc.sync.dma_start(b1v_sb[:], b1v[:])
            b2v_sb = cpool.tile([C, 1], F32)
            nc.sync.dma_start(b2v_sb[:], b2v[:])
            if apply_gb:
                g1m_sb = cpool.tile([T, C], F32)
                nc.sync.dma_start(g1m_sb[:], g1m[:])
                b1m_sb = cpool.tile([T, C], F32)
                nc.sync.dma_start(b1m_sb[:], b1m[:])
                g2m_sb = cpool.tile([T, C], F32)
                nc.sync.dma_start(g2m_sb[:], g2m[:])
                b2m_sb = cpool.tile([T, C], F32)
                nc.sync.dma_start(b2m_sb[:], b2m[:])

            # ---- phase 0: load all x, LN1 stats over all 128 batches ----
            xgs = []
            parts1 = p_parts1.tile([128, BL, 6], F32)
            for g in range(NG):
                xg = p_xg.tile([128, G, C], F32, tag="xg")
                nc.sync.dma_start(
                    xg[:], x_in[g * G : (g + 1) * G].rearrange("b t c -> t b c")
                )
                _bn_stats_blocks(
                    nc,
                    parts1[:],
                    g * G,
                    xg[:].rearrange("p g c -> p (g c)"),
                    G,
                )
                xgs.append(xg)

            sums1 = _ln_stats_from_parts(nc, p_st, parts1[:], BL, "ln1")
            tot1 = p_st.tile([128, 2 * BL], F32, tag="ln1_tot")
            nc.gpsimd.partition_all_reduce(
                tot1[:], sums1[:], 128, bass_isa.ReduceOp.add
            )
            mu1 = p_st.tile([128, BL], F32, tag="ln1_mu")
            nc.vector.tensor_scalar(mu1[:], tot1[:, 0:BL], 1.0 / NTC, None, OP.mult)
            ex2 = p_st.tile([128, BL], F32, tag="ln1_ex2")
            nc.vector.tensor_scalar(
                ex2[:], tot1[:, BL : 2 * BL], 1.0 / NTC, None, OP.mult
            )
            # var = ex2 - mu^2
            var1 = p_st.tile([128, BL], F32, tag="ln1_var")
            nc.vector.tensor_tensor(var1[:], mu1[:], mu1[:], OP.mult)
            nc.vector.tensor_tensor(var1[:], ex2[:], var1[:], OP.subtract)
            inv1 = _newton_rsqrt(nc, p_st, var1[:], BL, 1.0, 3, "ln1")

            h_eng = nc.gpsimd if H_NORM_ENGINE == "gpsimd" else nc.vector

            # ---- main loop: LN2 batches of SG2 groups ----
            for sb in range(NG // SG2):
                gs = [sb * SG2 + k for k in range(SG2)]
                tms = {}
                # stage A: h-norm + time-mix (+ x residual) for the batch
                for g in gs:
                    xg = xgs[g]
                    xg_flat = xg[:].rearrange("p g c -> p (g c)")
                    h = p_h.tile([128, FD], mmdt, tag="h")
                    for b in range(G):
                        col = g * G + b
                        h_eng.tensor_scalar(
                            h[:, b * C : (b + 1) * C],
                            xg[:, b, :],
                            mu1[:, col : col + 1],
                            inv1[:, col : col + 1],
                            OP.subtract,
                            OP.mult,
                        )
                    if apply_gb:
                        for b in range(G):
                            blk = h[:, b * C : (b + 1) * C]
                            nc.vector.tensor_tensor(blk, blk, g1m_sb[:], OP.mult)
                            nc.vector.tensor_tensor(blk, blk, b1m_sb[:], OP.add)
                    tm = p_tm.tile([128, FD], F32, tag="tm")
                    nc.tensor.matmul(
                        tm[:], wmT_sb[:], h[:], start=True, stop=False
                    )
                    nc.tensor.matmul(
                        tm[:], ident_sb[:], xg_flat, start=False, stop=True
                    )
                    tms[g] = tm

                # LN2 stats for the batch
                nb2 = SG2 * G
                parts2 = p_parts2.tile([128, nb2, 6], F32, tag="parts2")
                for k, g in enumerate(gs):
                    _bn_stats_blocks(nc, parts2[:], k * G, tms[g][:], G)
                # fold bt into the per-partition means (var parts unchanged)
                means2 = parts2[:].rearrange("p s (a b) -> p s a b", a=2, b=3)[
                    :, :, :, 1:2
                ].squeeze(3)
                nc.vector.tensor_scalar(
                    means2, means2, btv_sb[:, 0:1], None, OP.add
                )
                sums2 = _ln_stats_from_parts(nc, p_st, parts2[:], nb2, "ln2")
                tot2 = p_st.tile([128, 2 * nb2], F32, tag="ln2_tot")
                nc.gpsimd.partition_all_reduce(
                    tot2[:], sums2[:], 128, bass_isa.ReduceOp.add
                )
                mu2 = p_st.tile([128, nb2], F32, tag="ln2_mu")
                nc.vector.tensor_scalar(
                    mu2[:], tot2[:, 0:nb2], 1.0 / NTC, None, OP.mult
                )
                ex22 = p_st.tile([128, nb2], F32, tag="ln2_ex2")
                nc.vector.tensor_scalar(
                    ex22[:], tot2[:, nb2 : 2 * nb2], 1.0 / NTC, None, OP.mult
                )
                var2 = p_st.tile([128, nb2], F32, tag="ln2_var")
                nc.vector.tensor_tensor(var2[:], mu2[:], mu2[:], OP.mult)
                nc.vector.tensor_tensor(var2[:], ex22[:], var2[:], OP.subtract)
                inv2 = _newton_rsqrt(nc, p_st, var2[:], nb2, 0.9, 3, "ln2")
                # bias2 = (bt - mu2) * inv2   [per partition t, per batch col]
                bias2 = p_st.tile([128, nb2], F32, tag="ln2_bias")
                nc.vector.tensor_scalar(
                    bias2[:], mu2[:], btv_sb[:, 0:1], -1.0, OP.subtract, OP.mult
                )
                nc.vector.tensor_tensor(bias2[:], bias2[:], inv2[:], OP.mult)

                # stage B per group
                for k, g in enumerate(gs):
                    tm = tms[g]
                    x2 = p_x2.tile([128, FD], F32, tag="x2")
                    for b in range(G):
                        col = k * G + b
                        args = (
                            x2[:, b * C : (b + 1) * C],
                            tm[:, b * C : (b + 1) * C],
                        )
                        if b < X2_EVICT_ACT_BLOCKS:
                            nc.scalar.activation(
                                *args,
                                AF.Identity,
                                bias=bias2[:, col : col + 1],
                                scale=inv2[:, col : col + 1],
                            )
                        else:
                            nc.vector.tensor_scalar(
                                args[0],
                                args[1],
                                inv2[:, col : col + 1],
                                bias2[:, col : col + 1],
                                OP.mult,
                                OP.add,
                            )
                    if apply_gb:
                        for b in range(G):
                            blk = x2[:, b * C : (b + 1) * C]
                            nc.vector.tensor_tensor(blk, blk, g2m_sb[:], OP.mult)
                            nc.vector.tensor_tensor(blk, blk, b2m_sb[:], OP.add)

                    # transpose x2 -> channel-major
                    x2ct_ps = p_ctps.tile([128, FD], F32, tag="x2ct_ps")
                    for b in range(G):
                        nc.tensor.matmul(
                            x2ct_ps[:, b * C : (b + 1) * C],
                            x2[:, b * C : (b + 1) * C],
                            ident_sb[:],
                            is_transpose=True,
                            start=True,
                            stop=True,
                        )
                    x2ct = p_x2ct.tile([128, FD], mmdt, tag="x2ct")
                    nc.scalar.copy(x2ct[:], x2ct_ps[:])

                    # MLP
                    m1 = p_m1.tile([128, FD], F32, tag="m1")
                    nc.tensor.matmul(
                        m1[:], w1T_sb[:], x2ct[:], start=True, stop=True
                    )
                    gct = p_gct.tile([128, FD], mmdt, tag="gct")
                    nc.scalar.activation(
                        gct[:],
                        m1[:],
                        AF.Identity if SIM_NOGELU else AF.Gelu,
                        bias=b1v_sb[:, 0:1],
                        scale=1.0,
                    )
                    h2 = p_h2.tile([128, FD], F32, tag="h2")
                    nc.tensor.matmul(
                        h2[:], w2T_sb[:], gct[:], start=True, stop=True
                    )
                    # y_ct = h2 + b2  (exact-residual comes later in time-major)
                    outct = p_outct.tile([128, FD], F32, tag="outct")
                    nc.scalar.activation(
                        outct[:], h2[:], AF.Identity, bias=b2v_sb[:, 0:1], scale=1.0
                    )
                    # transpose back to time-major
                    outtm = p_ot.tile([128, FD], F32, tag="outtm")
                    for b in range(G):
                        nc.tensor.matmul(
                            outtm[:, b * C : (b + 1) * C],
                            outct[:, b * C : (b + 1) * C],
                            ident_sb[:],
                            is_transpose=True,
                            start=True,
                            stop=True,
                        )
                    # out = y_tm + x2 (time-major x2 is exact fp32)
                    outsb = p_outsb.tile([128, G, C], F32, tag="outsb")
                    nc.vector.scalar_tensor_tensor(
                        outsb[:].rearrange("p g c -> p (g c)"),
                        outtm[:],
                        0.0,
                        x2[:],
                        OP.bypass,
                        OP.add,
                    )
                    nc.sync.dma_start(
                        y_out[g * G : (g + 1) * G].rearrange("b t c -> t b c"),
                        outsb[:],
                    )
    nc.finalize()
    return nc


_NC_CACHE: dict = {}


def _get_nc(apply_gb: bool) -> bass.Bass:
    key = (apply_gb, H_NORM_ENGINE, MM_DTYPE, X2_EVICT_ACT_BLOCKS)
    if key not in _NC_CACHE:
        _NC_CACHE[key] = build_nc(apply_gb)
    return _NC_CACHE[key]


def kernel(x, ln1_g, ln1_b, ln2_g, ln2_b, Wt, bt, W1, b1, W2, b2, **kw):
    f = np.float32
    x = np.ascontiguousarray(x, dtype=f)
    Wt = np.asarray(Wt, dtype=f)
    bt = np.asarray(bt, dtype=f)
    W1 = np.asarray(W1, dtype=f)
    b1 = np.asarray(b1, dtype=f)
    W2 = np.asarray(W2, dtype=f)
    b2 = np.asarray(b2, dtype=f)
    ln1_g = np.asarray(ln1_g, dtype=f)
    ln1_b = np.asarray(ln1_b, dtype=f)
    ln2_g = np.asarray(ln2_g, dtype=f)
    ln2_b = np.asarray(ln2_b, dtype=f)

    trivial = (
        np.all(ln1_g == 1.0)
        and np.all(ln1_b == 0.0)
        and np.all(ln2_g == 1.0)
        and np.all(ln2_b == 0.0)
    )
    nc = _get_nc(not trivial)

    wmT_np = np.ascontiguousarray((Wt * np.tril(np.ones((T, T), f))).T)
    w1T_np = np.ascontiguousarray(W1.T)
    w2T_np = np.ascontiguousarray(W2.T)
    ident_np = np.eye(128, dtype=f)
    btv_np = np.ascontiguousarray(bt.reshape(T, 1))
    b1v_np = np.ascontiguousarray(b1.reshape(C, 1))
    b2v_np = np.ascontiguousarray(b2.reshape(C, 1))

    in_maps = []
    for i in range(NCORES):
        m = {
            "x_in": np.ascontiguousarray(x[i * BL : (i + 1) * BL]),
            "wmT": wmT_np,
            "w1T": w1T_np,
            "w2T": w2T_np,
            "ident": ident_np,
            "btv": btv_np,
            "b1v": b1v_np,
            "b2v": b2v_np,
        }
        if not trivial:
            m["g1m"] = np.ascontiguousarray(ln1_g)
            m["b1m"] = np.ascontiguousarray(ln1_b)
            m["g2m"] = np.ascontiguousarray(ln2_g)
            m["b2m"] = np.ascontiguousarray(ln2_b)
        in_maps.append(m)

    trace = bool(os.environ.get("MIXER_TRACE"))
    res = run_bass_kernel_spmd(
        nc, in_maps, core_ids=list(range(NCORES)), trace=trace
    )
    global LAST_RESULTS
    LAST_RESULTS = res
    out = np.concatenate(
        [res.results[i]["y_out"] for i in range(NCORES)], axis=0
    )
    return np.ascontiguousarray(out, dtype=f)


LAST_RESULTS = None


if __name__ == "__main__":
    np.random.seed(0)
    import reference

    inputs = {k: np.asarray(v) for k, v in reference.setup_inputs().items()}
    expected = np.asarray(reference.reference(**inputs))
    actual = kernel(**inputs)
    err = np.abs(actual - expected)
    denom = np.maximum(np.abs(expected), 1e-6)
    print("max abs err:", err.max())
    print("max rel err:", (err / denom).max())


# revision 31
# speedup vs baseline: 599.7775x; 599.7775x over previous
"""Trainium2 Bass kernel for nn_Mixer2dTriU (B=1024, T=128, C=128, fp32).

Data-parallel over 8 NeuronCores: 128 batches/core, groups of 4 batches
stacked along the free dim ([128, 512] tiles).

Per-batch math (reference):
    h  = LN_{T,C}(x) * g1 + b1            (g1=ones, b1=zeros per spec)
    tm = tril(Wt) @ h + bt[:, None]
    x2 = LN_{T,C}(tm + x) * g2 + b2
    y  = gelu(x2 @ W1.T + b1v) @ W2.T + b2v
    out = x2 + y

Device design:
  - LN stats via DVE bn_stats + gpsimd partition_all_reduce (cross-partition)
    + Newton rsqrt on DVE (ACT Sqrt table is low-precision and would thrash
    the Gelu LUT table set).
  - time-mix = one batched matmul lhsT=tril(Wt).T (fp32r) + exact I@x (fp32)
    residual accumulated in PSUM; bt folded into LN2 stats + evict bias.
  - LN2 normalize fused into the PSUM->SBUF evict on ScalarE:
    x2 = tm_psum * inv2 + (bt - mu2) * inv2   (per-batch scalars).
  - PE transposes x2 -> channel-major, MLP via static-weight matmuls (fp32r),
    gelu+b1 fused on ScalarE, final residual via one scalar_tensor_tensor,
    PE transposes back, ScalarE evict, contiguous DMA store.
"""

import os
import sys

for _p in ("/opt/trn_rl_repo",):
    if _p not in sys.path and os.path.isdir(_p):
        sys.path.insert(0, _p)

import numpy as np

import concourse.bacc as bacc
import concourse.bass as bass
import concourse.mybir as mybir
from concourse import bass_isa
from concourse.bass_utils import run_bass_kernel_spmd
from concourse.tile import TileContext

B, T, C = 1024, 128, 128
NCORES = 8
BL = B // NCORES          # 128 batches per core
G = 4                     # batches per group -> free dim 512
NG = BL // G              # 32 groups
SG2 = int(os.environ.get("MIXER_SG2", "2"))  # groups per LN2 stats batch
SG1 = 8                   # groups per LN1 stats supergroup
EPS = 1e-5
NTC = float(T * C)        # elements per LN block
FD = G * C                # 512

F32 = mybir.dt.float32
F16 = mybir.dt.float16
F32R = mybir.dt.float32r
AX = mybir.AxisListType
OP = mybir.AluOpType
AF = mybir.ActivationFunctionType

# Engine for the LN1 normalize (h = (x-mu)*inv): "gpsimd" or "vector"
H_NORM_ENGINE = os.environ.get("MIXER_HNORM_ENGINE", "vector")
# Matmul dtype for the W-matmuls: "f32r" or "f32"
MM_DTYPE = os.environ.get("MIXER_MM_DTYPE", "f32r")
# How many of the 4 x2-evict blocks go to ScalarE (rest on VectorE)
X2_EVICT_ACT_BLOCKS = int(os.environ.get("MIXER_X2_ACT_BLOCKS", "4"))
# CoreSim has no Gelu LUT; this swaps in Identity for sim-only validation.
SIM_NOGELU = bool(os.environ.get("MIXER_SIM_NOGELU"))


def _bn_stats_pairs(nc, parts_ap, pair0, in_3d_ap, nblk):
    """bn_stats over a column-interleaved PAIR of C-blocks: stream order
    (c0,b0),(c0,b1),(c1,b0)... makes bn_stats' even/odd halves exactly the
    two batches' full 128-element stats. parts_ap: [128, npairs, 6];
    in_3d_ap: [128, nblk, C]."""
    pf = parts_ap.rearrange("p s k -> p (s k)")
    for k in range(nblk // 2):
        pair = pair0 + k
        in_ap = in_3d_ap[:, 2 * k : 2 * k + 2, :].rearrange("p g c -> p c g")
        nc.vector.add_instruction(
            mybir.InstBNStats(
                name=nc.get_next_instruction_name(),
                ins=[nc.vector.lower_ap(in_ap, opt=False)],
                outs=[nc.vector.lower_ap(pf[:, pair * 6 : (pair + 1) * 6])],
            )
        )


def _newton_rsqrt(nc, pool, varr, n, y0, iters, tag):
    """inv = rsqrt(varr + EPS) on DVE, [128, n] tiles. varr is an SBUF AP.

    Seed y0 (python float) must satisfy |y0*sqrt(v+eps) - 1| < ~0.5 for all
    expected v; each Newton step squares the error.
    """
    y = pool.tile([128, n], F32, tag=f"{tag}_y")
    # seed: y1 = 1.5*y0 - 0.5*y0^3*(var+eps), eps folded into the constant
    nc.vector.tensor_scalar(
        y[:], varr, -0.5 * y0 ** 3, 1.5 * y0 - 0.5 * y0 ** 3 * EPS,
        OP.mult, OP.add,
    )
    t = pool.tile([128, n], F32, tag=f"{tag}_t")
    for _ in range(iters - 1):
        nc.vector.tensor_tensor(t[:], y[:], y[:], OP.mult)
        nc.vector.tensor_tensor(t[:], t[:], varr, OP.mult)
        nc.vector.tensor_scalar(
            t[:], t[:], -0.5, 1.5 - 0.5 * EPS, OP.mult, OP.add
        )
        nc.vector.tensor_tensor(y[:], y[:], t[:], OP.mult)
    return y


def _ln_stats_from_parts(nc, pool, parts_ap, nb, tag, btv_col=None):
    """Pair-mode parts [128, nb//2, 6] = (count, mean, 128*var) x (b0, b1).
    Returns sums tile [128, 2*nb]: cols 0:nb per-partition block sums,
    nb:2nb per-partition block sum-of-squares. btv_col ([P,1]) is added to
    the means first (time-mix bias folded into LN2 stats)."""
    means = parts_ap.rearrange("p s (a b) -> p s a b", a=2, b=3)[:, :, :, 1:2]
    means = means.squeeze(3).rearrange("p s t -> p (s t)")   # [128, nb]
    ctvs = parts_ap.rearrange("p s (a b) -> p s a b", a=2, b=3)[:, :, :, 2:3]
    ctvs = ctvs.squeeze(3).rearrange("p s t -> p (s t)")     # [128, nb]

    if btv_col is not None:
        nc.vector.tensor_scalar(means, means, btv_col, None, OP.add)
    msq = pool.tile([128, nb], F32, tag=f"{tag}_msq")
    nc.vector.tensor_tensor(msq[:], means, means, OP.mult)
    sums = pool.tile([128, 2 * nb], F32, tag=f"{tag}_sums")
    nc.vector.tensor_scalar(sums[:, 0:nb], means, 128.0, None, OP.mult)
    nc.vector.scalar_tensor_tensor(
        sums[:, nb : 2 * nb], msq[:], 128.0, ctvs, OP.mult, OP.add
    )
    return sums


def build_nc(apply_gb: bool) -> bass.Bass:
    nc = bacc.Bacc()

    x_in = nc.declare_dram_parameter("x_in", [BL, T, C], F32, isOutput=False)
    mmdt = F32R if MM_DTYPE == "f32r" else F32
    wmT = nc.declare_dram_parameter("wmT", [T, T], mmdt, isOutput=False)
    w1T = nc.declare_dram_parameter("w1T", [C, C], mmdt, isOutput=False)
    w2T = nc.declare_dram_parameter("w2T", [C, C], F16, isOutput=False)
    onesr = nc.declare_dram_parameter("onesr", [1, 128], F16, isOutput=False)
    b2rep = nc.declare_dram_parameter("b2rep", [1, FD], F16, isOutput=False)
    ident = nc.declare_dram_parameter("ident", [128, 128], F32, isOutput=False)
    ones_m = nc.declare_dram_parameter("ones_m", [128, 128], F32, isOutput=False)
    btv = nc.declare_dram_parameter("btv", [T, 1], F32, isOutput=False)
    b1v = nc.declare_dram_parameter("b1v", [C, 1], F32, isOutput=False)
    if apply_gb:
        g1m = nc.declare_dram_parameter("g1m", [T, C], F32, isOutput=False)
        b1m = nc.declare_dram_parameter("b1m", [T, C], F32, isOutput=False)
        g2m = nc.declare_dram_parameter("g2m", [T, C], F32, isOutput=False)
        b2m = nc.declare_dram_parameter("b2m", [T, C], F32, isOutput=False)
    y_out = nc.declare_dram_parameter("y_out", [BL, T, C], F32, isOutput=True)

    with TileContext(nc) as tc:
        with (
            tc.tile_pool(name="const", bufs=1) as cpool,
            tc.tile_pool(name="xg", bufs=2 * SG1 + 2) as p_xg,
            tc.tile_pool(name="h", bufs=4) as p_h,
            tc.tile_pool(name="x2", bufs=4) as p_x2,
            tc.tile_pool(name="x2ct", bufs=4) as p_x2ct,
            tc.tile_pool(name="gct", bufs=3) as p_gct,
            tc.tile_pool(name="outsb", bufs=4) as p_outsb,
            tc.tile_pool(name="ytm", bufs=3) as p_ytm,
            tc.tile_pool(name="stats", bufs=3) as p_st,
            tc.tile_pool(name="parts1", bufs=2) as p_parts1,
            tc.tile_pool(name="parts2", bufs=3) as p_parts2,
            tc.tile_pool(name="tmps", bufs=4, space="PSUM") as p_tm,
            tc.tile_pool(name="ctps", bufs=1, space="PSUM") as p_ctps,
            tc.tile_pool(name="m1ps", bufs=1, space="PSUM") as p_m1,
            tc.tile_pool(name="otps", bufs=1, space="PSUM") as p_ot,
            tc.tile_pool(name="stps", bufs=1, space="PSUM") as p_stp,
        ):
            # ---- constants ----
            wmT_sb = cpool.tile([T, T], mmdt)
            nc.sync.dma_start(wmT_sb[:], wmT[:])
            w1T_sb = cpool.tile([C, C], mmdt)
            nc.sync.dma_start(w1T_sb[:], w1T[:])
            w2T_sb = cpool.tile([C, C], F16)
            nc.sync.dma_start(w2T_sb[:], w2T[:])
            onesr_sb = cpool.tile([1, 128], F16)
            nc.sync.dma_start(onesr_sb[:], onesr[:])
            b2rep_sb = cpool.tile([1, FD], F16)
            nc.sync.dma_start(b2rep_sb[:], b2rep[:])
            ident_sb = cpool.tile([128, 128], F32)
            nc.sync.dma_start(ident_sb[:], ident[:])
            ones_sb = cpool.tile([128, 128], F32)
            nc.sync.dma_start(ones_sb[:], ones_m[:])
            btv_sb = cpool.tile([T, 1], F32)
            nc.sync.dma_start(btv_sb[:], btv[:])
            b1v_sb = cpool.tile([C, 1], F32)
            nc.sync.dma_start(b1v_sb[:], b1v[:])
            if apply_gb:
                g1m_sb = cpool.tile([T, C], F32)
                nc.sync.dma_start(g1m_sb[:], g1m[:])
                b1m_sb = cpool.tile([T, C], F32)
                nc.sync.dma_start(b1m_sb[:], b1m[:])
                g2m_sb = cpool.tile([T, C], F32)
                nc.sync.dma_start(g2m_sb[:], g2m[:])
                b2m_sb = cpool.tile([T, C], F32)
                nc.sync.dma_start(b2m_sb[:], b2m[:])

            # ---- software-pipelined main loop ----
            # Per LN2-batch (SG2 groups) slot m we emit:
            #   h-norm + timemix(m) -> LN2 stats chain(m) -> [next-sg loads]
            #   -> stage B(m-1).
            # Stage B of slot m runs while slot m+1's stats chain occupies
            # DVE/Pool, keeping PE/ACT dense despite in-order engine streams.
            h_eng = nc.gpsimd if H_NORM_ENGINE == "gpsimd" else nc.vector
            xgs = {}
            stats1 = {}   # sg -> (mu1, inv1)
            stats2 = {}   # slot -> (inv2, bias2)
            tms = {}      # g -> tm psum tile

            def emit_load_bn1(sg, k, parts1):
                g = sg * SG1 + k
                xg = p_xg.tile([128, G, C], F32, tag="xg")
                nc.sync.dma_start(
                    xg[:], x_in[g * G : (g + 1) * G].rearrange("b t c -> t b c")
                )
                _bn_stats_pairs(nc, parts1[:], k * (G // 2), xg[:], G)
                xgs[g] = xg

            def emit_chain1(sg, parts1):
                nb1 = SG1 * G
                sums1 = _ln_stats_from_parts(nc, p_st, parts1[:], nb1, "ln1")
                tot1 = p_stp.tile([128, 2 * nb1], F32, tag="stat_tot")
                nc.tensor.matmul(tot1[:], ones_sb[:], sums1[:], start=True, stop=True)
                muex1 = p_st.tile([128, 2 * nb1], F32, tag="ln1_muex")
                nc.vector.tensor_scalar(
                    muex1[:], tot1[:], 1.0 / NTC, None, OP.mult
                )
                mu1 = muex1[:, 0:nb1]
                var1 = p_st.tile([128, nb1], F32, tag="ln1_var")
                nc.vector.tensor_tensor(var1[:], mu1, mu1, OP.mult)
                nc.vector.tensor_tensor(
                    var1[:], muex1[:, nb1 : 2 * nb1], var1[:], OP.subtract
                )
                inv1 = _newton_rsqrt(nc, p_st, var1[:], nb1, 1.0, 2, "ln1")
                nmi1 = p_st.tile([128, nb1], F32, tag="ln1_nmi")
                nc.vector.tensor_tensor(nmi1[:], mu1[:], inv1[:], OP.mult)
                nc.vector.tensor_scalar(nmi1[:], nmi1[:], -1.0, None, OP.mult)
                stats1[sg] = (nmi1, inv1)

            def emit_stage_a(slot):
                sg, gs = slots[slot]
                nmi1, inv1 = stats1[sg]
                for g in gs:
                    xg = xgs[g]
                    h = p_h.tile([128, FD], mmdt, tag="h")
                    for b in range(G):
                        col = (g - sg * SG1) * G + b
                        if H_NORM_ENGINE == "scalar":
                            nc.scalar.activation(
                                h[:, b * C : (b + 1) * C],
                                xg[:, b, :],
                                AF.Identity,
                                bias=nmi1[:, col : col + 1],
                                scale=inv1[:, col : col + 1],
                            )
                        else:
                            nc.vector.tensor_scalar(
                                h[:, b * C : (b + 1) * C],
                                xg[:, b, :],
                                inv1[:, col : col + 1],
                                nmi1[:, col : col + 1],
                                OP.mult,
                                OP.add,
                            )
                    if apply_gb:
                        for b in range(G):
                            blk = h[:, b * C : (b + 1) * C]
                            nc.vector.tensor_tensor(blk, blk, g1m_sb[:], OP.mult)
                            nc.vector.tensor_tensor(blk, blk, b1m_sb[:], OP.add)
                    tm = p_tm.tile([128, FD], F32, tag="tm")
                    nc.tensor.matmul(tm[:], wmT_sb[:], h[:], start=True, stop=False)
                    nc.tensor.matmul(
                        tm[:],
                        ident_sb[:],
                        xg[:].rearrange("p g c -> p (g c)"),
                        start=False,
                        stop=True,
                    )
                    tms[g] = tm

            def emit_stats2(slot):
                sg, gs = slots[slot]
                nb2 = SG2 * G
                parts2 = p_parts2.tile([128, nb2 // 2, 6], F32, tag="parts2")
                for k, g in enumerate(gs):
                    _bn_stats_pairs(
                        nc,
                        parts2[:],
                        k * (G // 2),
                        tms[g][:].rearrange("p (g c) -> p g c", g=G),
                        G,
                    )
                sums2 = _ln_stats_from_parts(
                    nc, p_st, parts2[:], nb2, "ln2", btv_col=btv_sb[:, 0:1]
                )
                tot2 = p_stp.tile([128, 2 * nb2], F32, tag="stat_tot")
                nc.tensor.matmul(tot2[:], ones_sb[:], sums2[:], start=True, stop=True)
                muex2 = p_st.tile([128, 2 * nb2], F32, tag="ln2_muex")
                nc.vector.tensor_scalar(
                    muex2[:], tot2[:], 1.0 / NTC, None, OP.mult
                )
                mu2 = muex2[:, 0:nb2]
                var2 = p_st.tile([128, nb2], F32, tag="ln2_var")
                nc.vector.tensor_tensor(var2[:], mu2, mu2, OP.mult)
                nc.vector.tensor_tensor(
                    var2[:], muex2[:, nb2 : 2 * nb2], var2[:], OP.subtract
                )
                inv2 = _newton_rsqrt(nc, p_st, var2[:], nb2, 0.928, 3, "ln2")
                bias2 = p_st.tile([128, nb2], F32, tag="ln2_bias")
                nc.vector.tensor_scalar(
                    bias2[:], mu2, btv_sb[:, 0:1], -1.0, OP.subtract, OP.mult
                )
                nc.vector.tensor_tensor(bias2[:], bias2[:], inv2[:], OP.mult)
                stats2[slot] = (inv2, bias2)

            def emit_stage_b(slot):
                sg, gs = slots[slot]
                inv2, bias2 = stats2[slot]
                for k, g in enumerate(gs):
                    tm = tms.pop(g)
                    x2 = p_x2.tile([128, FD], F32, tag="x2")
                    for b in range(G):
                        col = k * G + b
                        args = (
                            x2[:, b * C : (b + 1) * C],
                            tm[:, b * C : (b + 1) * C],
                        )
                        if b < X2_EVICT_ACT_BLOCKS:
                            nc.scalar.activation(
                                *args,
                                AF.Identity,
                                bias=bias2[:, col : col + 1],
                                scale=inv2[:, col : col + 1],
                            )
                        else:
                            nc.vector.tensor_scalar(
                                args[0],
                                args[1],
                                inv2[:, col : col + 1],
                                bias2[:, col : col + 1],
                                OP.mult,
                                OP.add,
                            )
                    if apply_gb:
                        for b in range(G):
                            blk = x2[:, b * C : (b + 1) * C]
                            nc.vector.tensor_tensor(blk, blk, g2m_sb[:], OP.mult)
                            nc.vector.tensor_tensor(blk, blk, b2m_sb[:], OP.add)

                    x2ct_ps = p_ctps.tile([128, FD], F32, tag="x2ct_ps")
                    for b in range(G):
                        nc.tensor.matmul(
                            x2ct_ps[:, b * C : (b + 1) * C],
                            x2[:, b * C : (b + 1) * C],
                            ident_sb[:],
                            is_transpose=True,
                            start=True,
                            stop=True,
                        )
                    x2ct = p_x2ct.tile([128, FD], mmdt, tag="x2ct")
                    nc.scalar.copy(x2ct[:], x2ct_ps[:])

                    m1 = p_m1.tile([128, FD], F32, tag="m1")
                    nc.tensor.matmul(
                        m1[:], w1T_sb[:], x2ct[:], start=True, stop=True
                    )
                    gct = p_gct.tile([128, FD], F16, tag="gct")
                    nc.scalar.activation(
                        gct[:],
                        m1[:],
                        AF.Identity if SIM_NOGELU else AF.Gelu,
                        bias=b1v_sb[:, 0:1],
                        scale=1.0,
                    )
                    # y_tm = gct.T @ W2.T + b2 directly in time-major:
                    # lhsT = gct block [o, t], rhs = w2T [o, o2]; b2 enters as
                    # a K=1 rank-1 (ones x b2rep) clearing the bank first.
                    outtm = p_ot.tile([128, FD], F32, tag="outtm")
                    nc.tensor.matmul(
                        outtm[:], onesr_sb[:], b2rep_sb[:], start=True, stop=False
                    )
                    for b in range(G):
                        nc.tensor.matmul(
                            outtm[:, b * C : (b + 1) * C],
                            gct[:, b * C : (b + 1) * C],
                            w2T_sb[:],
                            start=False,
                            stop=(b == G - 1),
                        )
                    ytm = p_ytm.tile([128, FD], F32, tag="ytm")
                    nc.scalar.copy(ytm[:], outtm[:])
                    outsb = p_outsb.tile([128, G, C], F32, tag="outsb")
                    nc.gpsimd.tensor_tensor(
                        outsb[:].rearrange("p g c -> p (g c)"),
                        ytm[:],
                        x2[:],
                        OP.add,
                    )
                    nc.sync.dma_start(
                        y_out[g * G : (g + 1) * G].rearrange("b t c -> t b c"),
                        outsb[:],
                    )

            # slot table: NG//SG2 LN2 batches
            slots = []
            for sg in range(NG // SG1):
                for sb in range(SG1 // SG2):
                    gs = [sg * SG1 + sb * SG2 + k for k in range(SG2)]
                    slots.append((sg, gs))
            per_sg = SG1 // SG2

            parts1_tiles = {}
            parts1_tiles[0] = p_parts1.tile([128, SG1 * G // 2, 6], F32, tag="parts1", name="parts1")
            for k in range(SG1):
                emit_load_bn1(0, k, parts1_tiles[0])
            emit_chain1(0, parts1_tiles[0])

            nsg = NG // SG1
            for m, (sg, gs) in enumerate(slots):
                emit_stage_a(m)
                emit_stats2(m)
                # interleave next supergroup's loads + bn1
                sb_i = m % per_sg
                if sg + 1 < nsg:
                    if sb_i == 0:
                        parts1_tiles[sg + 1] = p_parts1.tile(
                            [128, SG1 * G // 2, 6], F32, tag="parts1", name="parts1"
                        )
                    kper = (SG1 + per_sg - 1) // per_sg
                    for k in range(sb_i * kper, min((sb_i + 1) * kper, SG1)):
                        emit_load_bn1(sg + 1, k, parts1_tiles[sg + 1])
                    if sb_i == per_sg - 1:
                        emit_chain1(sg + 1, parts1_tiles[sg + 1])
                if m >= 1:
                    emit_stage_b(m - 1)
            emit_stage_b(len(slots) - 1)
    nc.finalize()
    return nc


_NC_CACHE: dict = {}


def _get_nc(apply_gb: bool) -> bass.Bass:
    key = (apply_gb, H_NORM_ENGINE, MM_DTYPE, X2_EVICT_ACT_BLOCKS)
    if key not in _NC_CACHE:
        _NC_CACHE[key] = build_nc(apply_gb)
    return _NC_CACHE[key]


def kernel(x, ln1_g, ln1_b, ln2_g, ln2_b, Wt, bt, W1, b1, W2, b2, **kw):
    f = np.float32
    x = np.ascontiguousarray(x, dtype=f)
    Wt = np.asarray(Wt, dtype=f)
    bt = np.asarray(bt, dtype=f)
    W1 = np.asarray(W1, dtype=f)
    b1 = np.asarray(b1, dtype=f)
    W2 = np.asarray(W2, dtype=f)
    b2 = np.asarray(b2, dtype=f)
    ln1_g = np.asarray(ln1_g, dtype=f)
    ln1_b = np.asarray(ln1_b, dtype=f)
    ln2_g = np.asarray(ln2_g, dtype=f)
    ln2_b = np.asarray(ln2_b, dtype=f)

    trivial = (
        np.all(ln1_g == 1.0)
        and np.all(ln1_b == 0.0)
        and np.all(ln2_g == 1.0)
        and np.all(ln2_b == 0.0)
    )
    nc = _get_nc(not trivial)

    wmT_np = np.ascontiguousarray((Wt * np.tril(np.ones((T, T), f))).T)
    w1T_np = np.ascontiguousarray(W1.T)
    w2T_np = np.ascontiguousarray(W2.T.astype(np.float16))
    onesr_np = np.ones((1, 128), np.float16)
    b2rep_np = np.ascontiguousarray(
        np.tile(b2.astype(np.float16), G).reshape(1, G * C)
    )
    ident_np = np.eye(128, dtype=f)
    ones_np = np.ones((128, 128), f)
    btv_np = np.ascontiguousarray(bt.reshape(T, 1))
    b1v_np = np.ascontiguousarray(b1.reshape(C, 1))

    in_maps = []
    for i in range(NCORES):
        m = {
            "x_in": np.ascontiguousarray(x[i * BL : (i + 1) * BL]),
            "wmT": wmT_np,
            "w1T": w1T_np,
            "w2T": w2T_np,
            "onesr": onesr_np,
            "b2rep": b2rep_np,
            "ident": ident_np,
            "ones_m": ones_np,
            "btv": btv_np,
            "b1v": b1v_np,
        }
        if not trivial:
            m["g1m"] = np.ascontiguousarray(ln1_g)
            m["b1m"] = np.ascontiguousarray(ln1_b)
            m["g2m"] = np.ascontiguousarray(ln2_g)
            m["b2m"] = np.ascontiguousarray(ln2_b)
        in_maps.append(m)

    trace = bool(os.environ.get("MIXER_TRACE"))
    res = run_bass_kernel_spmd(
        nc, in_maps, core_ids=list(range(NCORES)), trace=trace
    )
    global LAST_RESULTS
    LAST_RESULTS = res
    out = np.concatenate(
        [res.results[i]["y_out"] for i in range(NCORES)], axis=0
    )
    return np.ascontiguousarray(out, dtype=f)


LAST_RESULTS = None


if __name__ == "__main__":
    np.random.seed(0)
    import reference

    inputs = {k: np.asarray(v) for k, v in reference.setup_inputs().items()}
    expected = np.asarray(reference.reference(**inputs))
    actual = kernel(**inputs)
    err = np.abs(actual - expected)
    denom = np.maximum(np.abs(expected), 1e-6)
    print("max abs err:", err.max())
    print("max rel err:", (err / denom).max())
